# revision 2
# baseline (speedup 1.0000x reference)
"""DetailPooling Trainium2 Bass kernel, v5.1.

Per group (G=2 images, 2 h-tiles merged into [128, 2, G, W] tiles):
  Pool : s = x[j-1]+x[j] (f32 in, f16 out) interior + edge cols,
         num tail columns (rat*x)
  DVE  : t = s[j]+s[j+1] (TT @2x), e4 = Em+a' (@4x),
         rat = e4 * recip1(den) (custom 6-stage: BITWISE_NOT seed +
         1 Newton + in1 mult, ~0.17% rel), num head columns (TT f32, 1x),
         out drain (PSUM->SBUF)
  PE   : d = I@x_f32r - V@t - H@t_nbr (3 mm/tile),
         den = B@e4 + B@e4_sh + edge + halo (3 mm/tile),
         out pool (4 mm, col-split tile_position)
  ACT  : |d| (merged 2-bank PSUM -> SBUF f16), Ln(+1e-6),
         Exp(scale=4|lam|, bias=ln 1/4)
"""

import os
import numpy as np

N_CORES = 8
B, C, H, W = 16, 64, 256, 256
B_LOC = B // N_CORES          # 2 samples per core
P = B_LOC * C                 # 128 images per core
HO, WO = H // 2, W // 2
G = 2                         # images per group
N_GROUPS = P // G             # 64

_cache = {}

BUFS = int(os.environ.get("KERNEL_BUFS", "8"))
BUFS_D = int(os.environ.get("KERNEL_BUFS_D", "1"))
BUFS_DN = int(os.environ.get("KERNEL_BUFS_DN", "2"))
BUFS_OP = int(os.environ.get("KERNEL_BUFS_OP", "2"))
# columns of W handled by DVE for the num multiply; rest go to Pool
NUM_SPLIT_W = int(os.environ.get("KERNEL_NUM_SPLIT", "168"))


def _register_recip1_mul():
    """out = in1 * approx(1/in0): BITWISE_NOT exponent-flip seed + one
    inline Newton pass + elementwise in1 multiply (6 stages, ~0.17% rel
    err for normal positive in0)."""
    import concourse.dve_ops as dve_ops

    if hasattr(dve_ops, "RECIP1_MUL_ANT"):
        return dve_ops.RECIP1_MUL_ANT
    from concourse.dve_spec import AluOp, Bin, C0, C1, Spec, Src0, Src1

    def _ref(in0, in1, c0, c1, c2):
        not_x = (~in0.view(np.int32)).view(np.float32)
        y0 = not_x * np.float32(c0)
        return (in1 * y0) * (np.float32(c1) - in0 * y0)

    _nx = Bin(AluOp.BITWISE_NOT, Src0, Src0)
    _y0 = _nx * C0
    spec = Spec(body=(Src1 * _y0) * (C1 - Src0 * _y0), reference=_ref)
    op = dve_ops.DveOp(
        "RECIP1_MUL_ANT", spec, subdim=False,
        uops_sha={"v3": "c86b792ab9e25941", "v4": "630fa4edde6b706f"})
    dve_ops.OPS.append(op)
    dve_ops.CUSTOM_DVE_SPECS[op.name] = spec
    dve_ops._SUB_OPCODE_FOR_NAME[op.name] = (
        max(dve_ops._SUB_OPCODE_FOR_NAME.values()) + 1)
    dve_ops.RECIP1_MUL_ANT = op
    return op


RECIP1_MUL_C = {"s0": -0.23549792, "s1": 2.0017324}


def _build(cfg=None, rep=1, probe=None):
    import concourse.mybir as mybir
    from concourse import bacc, tile

    recip1_mul = _register_recip1_mul()

    # abs/ln/exp all live in natural_log_exp_and_others; blank the competing
    # sets so bacc's placement never inserts 1.3us table swaps.
    if not getattr(bacc, "_nlx_tables_patch", False):
        _orig_tables = bacc.get_activation_tables

        def _tables_nlx(arch):
            t = _orig_tables(arch)
            keep = {"natural_log_exp_and_others"}
            return {n: (f if n in keep else set()) for n, f in t.items()}

        bacc.get_activation_tables = _tables_nlx
        bacc._nlx_tables_patch = True

    f32 = mybir.dt.float32
    f32r = mybir.dt.float32r
    bf16 = mybir.dt.bfloat16
    f16 = mybir.dt.float16
    i32 = mybir.dt.int32
    Alu = mybir.AluOpType
    Act = mybir.ActivationFunctionType

    nc = bacc.Bacc("TRN2", target_bir_lowering=False, debug=False,
                   num_devices=N_CORES)
    x_ap = nc.dram_tensor("x", [P, H * W], f32, kind="ExternalInput").ap()
    lam_ap = nc.dram_tensor("lam", [1, 1], f32, kind="ExternalInput").ap()
    alpha_ap = nc.dram_tensor("alpha", [1, 1], f32,
                              kind="ExternalInput").ap()
    out_ap = nc.dram_tensor("out", [P, HO * WO], f32,
                            kind="ExternalOutput").ap()

    x4 = x_ap.bitcast(f32r).rearrange("p (t h w) -> h t p w", t=2, w=W)
    od = out_ap.rearrange("p (h w) -> p h w", w=WO)   # [128 img, 128, 128]

    with tile.TileContext(nc) as tc:
        with tc.tile_pool(name="cpool", bufs=1) as cpool, \
             tc.tile_pool(name="pool", bufs=1) as pool, \
             tc.tile_pool(name="ppool", bufs=1, space="PSUM") as ppool:
            # ---- scalars: 4|lam|, (|alpha|+1e-8)/4, ln(1/4), 1e-6
            sc_row = cpool.tile([1, 8], f32)
            nc.sync.dma_start(sc_row[0:1, 0:1], lam_ap)
            nc.sync.dma_start(sc_row[0:1, 1:2], alpha_ap)
            nc.scalar.activation(sc_row[0:1, 2:3], sc_row[0:1, 0:1],
                                 Act.Abs, scale=4.0)         # 4|lam|
            nc.scalar.activation(sc_row[0:1, 3:4], sc_row[0:1, 1:2],
                                 Act.Abs)                    # |alpha|
            nc.vector.tensor_scalar(sc_row[0:1, 4:5], sc_row[0:1, 3:4],
                                    1e-8, 0.25, Alu.add,
                                    Alu.mult)                # (|alpha|+1e-8)/4
            scal = cpool.tile([128, 8], f32)
            nc.gpsimd.partition_broadcast(scal[:, :], sc_row[0:1, :])
            la4 = scal[:, 2:3]     # 4|lam|
            al4 = scal[:, 4:5]     # (|alpha|+1e-8)/4
            lnq = cpool.tile([128, 1], f32)
            nc.vector.memset(lnq[:], float(np.log(0.25)))
            eps6 = cpool.tile([128, 1], f32)
            nc.vector.memset(eps6[:], 1e-6)

            # ---- stationary matrices (bf16 [128,128], lhsT layout [k, m])
            Dm = cpool.tile([128, 128], i32)
            nc.gpsimd.iota(Dm[:], pattern=[[1, 128]], base=0,
                           channel_multiplier=-1)            # f - p
            D2 = cpool.tile([128, 128], i32)
            nc.gpsimd.iota(D2[:], pattern=[[-2, 128]], base=0,
                           channel_multiplier=1)             # p - 2f

            def eqm(dst, src, val):
                nc.vector.tensor_scalar(dst, src, float(val), None,
                                        Alu.is_equal)

            e0 = cpool.tile([128, 128], bf16)
            ep1 = cpool.tile([128, 128], bf16)
            em1 = cpool.tile([128, 128], bf16)
            eqm(e0[:], Dm[:], 0)       # k == m
            eqm(ep1[:], Dm[:], 1)      # m == k+1
            eqm(em1[:], Dm[:], -1)     # m == k-1

            Pi = cpool.tile([128, 128], i32)
            nc.gpsimd.iota(Pi[:], pattern=[[0, 128]], base=0,
                           channel_multiplier=1)
            Fi = cpool.tile([128, 128], i32)
            nc.gpsimd.iota(Fi[:], pattern=[[1, 128]], base=0,
                           channel_multiplier=0)
            rm0 = cpool.tile([128, 128], bf16)
            rm127 = cpool.tile([128, 128], bf16)
            cm0 = cpool.tile([128, 128], bf16)
            cm127 = cpool.tile([128, 128], bf16)
            eqm(rm0[:], Pi[:], 0)
            eqm(rm127[:], Pi[:], 127)
            eqm(cm0[:], Fi[:], 0)
            eqm(cm127[:], Fi[:], 127)
            c00 = cpool.tile([128, 128], bf16)       # 1 at [0, 0]
            c1717 = cpool.tile([128, 128], bf16)     # 1 at [127, 127]
            nc.vector.tensor_tensor(c00[:], rm0[:], cm0[:], Alu.mult)
            nc.vector.tensor_tensor(c1717[:], rm127[:], cm127[:], Alu.mult)

            # NV = -(1/16) * ([1,2,1] tridiag); NV0 top-replicate, NV1 bottom
            NV0 = cpool.tile([128, 128], bf16)
            NV1 = cpool.tile([128, 128], bf16)
            vtmp = cpool.tile([128, 128], bf16)
            nc.vector.tensor_tensor(vtmp[:], ep1[:], em1[:], Alu.add)
            nc.vector.scalar_tensor_tensor(vtmp[:], e0[:], 2.0, vtmp[:],
                                           Alu.mult, Alu.add)
            nc.vector.scalar_tensor_tensor(NV0[:], c00[:], 1.0, vtmp[:],
                                           Alu.mult, Alu.add)
            nc.vector.tensor_scalar_mul(NV0[:], NV0[:], -1.0 / 16.0)
            nc.vector.scalar_tensor_tensor(NV1[:], c1717[:], 1.0, vtmp[:],
                                           Alu.mult, Alu.add)
            nc.vector.tensor_scalar_mul(NV1[:], NV1[:], -1.0 / 16.0)
            # halo fixes: NH0[k=0, m=127] (tile0 <- tile1), NH1[k=127, m=0]
            NH0 = cpool.tile([128, 128], bf16)
            NH1 = cpool.tile([128, 128], bf16)
            nc.vector.tensor_tensor(NH0[:], rm0[:], cm127[:], Alu.mult)
            nc.vector.tensor_scalar_mul(NH0[:], NH0[:], -1.0 / 16.0)
            nc.vector.tensor_tensor(NH1[:], rm127[:], cm0[:], Alu.mult)
            nc.vector.tensor_scalar_mul(NH1[:], NH1[:], -1.0 / 16.0)
            # B: den vertical 2-tap: k in {m, m+1}
            B0 = cpool.tile([128, 128], bf16)
            B1 = cpool.tile([128, 128], bf16)
            nc.vector.tensor_tensor(B0[:], e0[:], em1[:], Alu.add)
            nc.vector.tensor_tensor(B1[:], B0[:], c1717[:], Alu.add)
            HB = cpool.tile([128, 128], bf16)
            nc.vector.tensor_tensor(HB[:], rm0[:], cm127[:], Alu.mult)
            # S2: final vertical stride-2 pool, k in {2m, 2m+1}
            S2 = cpool.tile([128, 64], bf16)
            s2a = cpool.tile([128, 64], bf16)
            eqm(S2[:], D2[:, 0:64], 0)
            eqm(s2a[:], D2[:, 0:64], 1)
            nc.vector.tensor_tensor(S2[:], S2[:], s2a[:], Alu.add)
            # identity in f32r for the I@x matmul
            If32r = cpool.tile([128, 128], f32r)
            eqm(If32r[:], Dm[:], 0)

            cw = NUM_SPLIT_W
            # software-pipelined with stage skew: consecutive ops on each
            # engine belong to different groups, so same-engine result
            # handoffs never stall. Stage offsets (group = i - off):
            #   dma/s/t/d/abs: 0, ln: 1, exp: 2, e4/den: 3, rat/num: 4,
            #   out/drain/dma-out: 5
            SKEW = 5
            tiles = {}
            n_iters = rep * N_GROUPS + SKEW

            def live(k):
                return 0 <= k < rep * N_GROUPS

            for i in range(n_iters):
                if live(i):
                    tiles[i] = T = {}
                    T["xm"] = pool.tile([128, 2, G, W], f32r, tag="x",
                                        name="xm", bufs=BUFS)
                    T["xv"] = T["xm"][:].bitcast(f32)
                    T["sm"] = pool.tile([128, 2, G, W + 2], f16, tag="s",
                                        name="sm", bufs=3)
                    T["tm"] = pool.tile([128, 2, G, W], f16, tag="t",
                                        name="tm", bufs=3)
                    img0 = G * (i % N_GROUPS)
                    # ---- DMA in: one per h-tile
                    for t in range(2):
                        nc.sync.dma_start(T["xm"][:, t],
                                          x4[:, t, img0:img0 + G, :])
                    # ---- Pool: s[j] = x[j-1] + x[j], interior + edges
                    nc.gpsimd.tensor_tensor(
                        T["sm"][:, :, :, 1:256], T["xv"][:, :, :, 0:255],
                        T["xv"][:, :, :, 1:256], Alu.add)
                    nc.gpsimd.tensor_tensor(
                        T["sm"][:, :, :, 0:257:256],
                        T["xv"][:, :, :, 0:256:255],
                        T["xv"][:, :, :, 0:256:255], Alu.add)
                    # ---- DVE: t[j] = s[j] + s[j+1]
                    nc.vector.tensor_tensor(
                        T["tm"][:], T["sm"][:, :, :, 0:256],
                        T["sm"][:, :, :, 1:257], Alu.add)

                # ---- PE: den(i-3) = 2x2 stride-1 sum of e4
                if live(i - 3):
                    T = tiles[i - 3]
                    T["e4m"] = pool.tile([128, 2, G, W], f16, tag="e4",
                                         name="e4m", bufs=3)
                    T["denps"] = ppool.tile([128, 2, G, W], f32, tag="dn",
                                            name="denps", bufs=BUFS_DN)
                    nc.vector.tensor_scalar(T["e4m"][:], T["Em"][:], al4,
                                            None, Alu.add)
                    e4m, denps = T["e4m"], T["denps"]
                    for t in range(2):
                        Bm = B0 if t == 0 else B1
                        nc.tensor.matmul(denps[:, t], Bm[:], e4m[:, t],
                                         start=True, stop=False)
                        nc.tensor.matmul(denps[:, t, :, 0:255], Bm[:],
                                         e4m[:, t, :, 1:256],
                                         start=False, stop=False)
                        last = t != 0
                        nc.tensor.matmul(denps[:, t, :, 255:256], Bm[:],
                                         e4m[:, t, :, 255:256],
                                         start=False, stop=last)
                        if t == 0:
                            nc.tensor.matmul(denps[:, 0], HB[:], e4m[:, 1],
                                             start=False, stop=False)
                            nc.tensor.matmul(denps[:, 0, :, 0:255], HB[:],
                                             e4m[:, 1, :, 1:256],
                                             start=False, stop=False)
                            nc.tensor.matmul(denps[:, 0, :, 255:256],
                                             HB[:], e4m[:, 1, :, 255:256],
                                             start=False, stop=True)

                # ---- PE: d(i) = x - (1/16) vblur(t)   (PSUM fp32)
                if live(i):
                    T = tiles[i]
                    T["dps"] = ppool.tile([128, 2, G, W], f32, tag="d",
                                          name="dps", bufs=BUFS_D)
                    dps, xm, tm = T["dps"], T["xm"], T["tm"]
                    for t in range(2):
                        NV = NV0 if t == 0 else NV1
                        NH = NH0 if t == 0 else NH1
                        nc.tensor.matmul(dps[:, t], If32r[:], xm[:, t],
                                         start=True, stop=False)
                        nc.tensor.matmul(dps[:, t], NV[:], tm[:, t],
                                         start=False, stop=False)
                        nc.tensor.matmul(dps[:, t], NH[:], tm[:, 1 - t],
                                         start=False, stop=True)

                # ---- PE: out(i-5) 2x2 stride-2 pool
                if live(i - 5):
                    T = tiles[i - 5]
                    T["ops"] = ppool.tile([128, G, WO], f32, tag="op",
                                          name="ops", bufs=BUFS_OP)
                    numm, ops = T["numm"], T["ops"]
                    nc.tensor.matmul(ops[0:64, :, :], S2[:],
                                     numm[:, 0, :, 0:256:2],
                                     start=True, stop=False,
                                     tile_position=(0, 0))
                    nc.tensor.matmul(ops[64:128, :, :], S2[:],
                                     numm[:, 1, :, 0:256:2],
                                     start=True, stop=False,
                                     tile_position=(0, 64))
                    nc.tensor.matmul(ops[0:64, :, :], S2[:],
                                     numm[:, 0, :, 1:256:2],
                                     start=False, stop=True,
                                     tile_position=(0, 0))
                    nc.tensor.matmul(ops[64:128, :, :], S2[:],
                                     numm[:, 1, :, 1:256:2],
                                     start=False, stop=True,
                                     tile_position=(0, 64))

                # ---- ACT: abs(i), ln(i-1), exp(i-2)
                if live(i):
                    T = tiles[i]
                    T["adm"] = pool.tile([128, 2, G, W], f16, tag="ad",
                                         name="adm", bufs=3)
                    nc.scalar.activation(T["adm"][:], T["dps"][:], Act.Abs)
                if live(i - 1):
                    T = tiles[i - 1]
                    T["ym"] = pool.tile([128, 2, G, W], f16, tag="y",
                                        name="ym", bufs=3)
                    nc.scalar.activation(T["ym"][:], T["adm"][:], Act.Ln,
                                         bias=eps6[:])
                if live(i - 2):
                    T = tiles[i - 2]
                    T["Em"] = pool.tile([128, 2, G, W], f16, tag="E",
                                        name="Em", bufs=3)
                    nc.scalar.activation(T["Em"][:], T["ym"][:], Act.Exp,
                                         scale=la4, bias=lnq[:])

                # ---- DVE: e4(i-3), rat(i-4), num(i-4), drain(i-5)
                if live(i - 4):
                    T = tiles[i - 4]
                    T["ratm"] = pool.tile([128, 2, G, W], f16, tag="ra",
                                          name="ratm", bufs=2)
                    T["numm"] = pool.tile([128, 2, G, W], f16, tag="nm",
                                          name="numm", bufs=3)
                    nc.vector._custom_dve(
                        recip1_mul,
                        out=T["ratm"][:].rearrange("p t g w -> p (t g) w"),
                        in0=T["denps"][:].rearrange("p t g w -> p (t g) w"),
                        in1=T["e4m"][:].rearrange("p t g w -> p (t g) w"),
                        s0=RECIP1_MUL_C["s0"], s1=RECIP1_MUL_C["s1"])
                    nc.vector.tensor_tensor(
                        T["numm"][:, :, :, 0:cw], T["ratm"][:, :, :, 0:cw],
                        T["xv"][:, :, :, 0:cw], Alu.mult)
                    nc.gpsimd.tensor_tensor(
                        T["numm"][:, :, :, cw:W], T["ratm"][:, :, :, cw:W],
                        T["xv"][:, :, :, cw:W], Alu.mult)
                if live(i - 5):
                    T = tiles.pop(i - 5)
                    T["outsb"] = pool.tile([128, G, WO], f32, tag="o",
                                           name="outsb", bufs=2)
                    nc.vector.tensor_scalar(T["outsb"][:], T["ops"][:],
                                            1.0, None, Alu.mult)
                    img0 = G * ((i - 5) % N_GROUPS)
                    nc.sync.dma_start(
                        od[img0:img0 + G, :, :].rearrange("p h w -> h p w"),
                        T["outsb"][:])
    nc.compile()
    return nc


def _get_nc():
    if "nc" not in _cache:
        _cache["nc"] = _build()
    return _cache["nc"]


def kernel(x, lam, alpha):
    if not int(os.environ.get("KERNEL_TRACE", "0")):
        os.environ["BASS_NEVER_TRACE"] = "1"
    jp = os.environ.get("JAX_PLATFORMS")
    if jp and "axon" not in jp:
        del os.environ["JAX_PLATFORMS"]
    import concourse.bass_utils as bass_utils

    x = np.ascontiguousarray(np.asarray(x, dtype=np.float32))
    lam = np.asarray(lam, dtype=np.float32).reshape(1, 1)
    alpha = np.asarray(alpha, dtype=np.float32).reshape(1, 1)
    assert x.shape == (B, C, H, W)

    nc = _get_nc()
    in_maps = []
    for i in range(N_CORES):
        shard = x[i * B_LOC:(i + 1) * B_LOC].reshape(P, H * W)
        in_maps.append({"x": np.ascontiguousarray(shard),
                        "lam": lam, "alpha": alpha})

    res = bass_utils.run_bass_kernel_spmd(
        nc, in_maps, core_ids=list(range(N_CORES)),
        trace=bool(int(os.environ.get("KERNEL_TRACE", "0"))))
    _cache["last_results"] = res

    out = np.empty((B, C, HO, WO), dtype=np.float32)
    for i in range(N_CORES):
        out[i * B_LOC:(i + 1) * B_LOC] = \
            res.results[i]["out"].reshape(B_LOC, C, HO, WO)
    return out
